# revision 18
# baseline (speedup 1.0000x reference)
"""YOLO-style detection layer on 8 Trainium2 NeuronCores (Bass/Tile).

Reference computation (per image):
  h = leaky_relu(conv3x3(x, conv_w) + conv_b, 0.1)          # [1024, 19, 19]
  o = conv1x1(h, detect_w) + detect_b                       # [255, 19, 19]
  per (pos, anchor): sigmoids, grid offsets, exp*anchor, max/argmax over 80
  out [B, 1083, 6] = (score, xc, yc, w, h, label)

Sharding: pure data parallel — batch 64 split 8 per core; weights replicated.

Implementation notes:
  - conv3x3 = 36 accumulating PE matmuls (9 taps x 4 ci-chunks) per co-chunk,
    all in fp16 (1 cyc/row vs fp32's 4): the only accuracy-critical output is
    the per-box argmax over 80 class scores, and fp16 matmul (11-bit mantissa,
    like tf32) flips ~34/69k near-tie labels -> rel err ~1.6e-2 < the 2e-2
    gate. (fp32 = 0 flips but 4x the PE time; bf16 = ~312 flips = fail;
    fp32r hits walrus ISA-check failures on odd/strided moving APs.)
  - each tap's moving operand is a fully CONTIGUOUS window: for dx != 0 taps
    the x tile is pre-shifted by one column (gpsimd copy, zero edge col) so
    the PE streams [ny*19] contiguous rows; strided 2-D windows cost an extra
    ~6-17ns/matmul on the AP sequencer (~13us/kernel). Rows only trimmed in
    y (ny = 19-|dy|); the zero edge columns make the x-edge taps exact.
  - the center tap goes first in each PSUM accumulation group so every
    element is overwritten before partial-region taps accumulate.
  - conv1x1 computed transposed: out[pos, 255] = h[cmid, pos].T @ w2t[cmid,
    255], positions on partitions, as a 2-TERM fp16 split: ACT writes
    h16 = Prelu(ps)+b1 and h16b = 2^-10 * h16 (exact: Prelu is positively
    homogeneous, so scale/bias-scaled ACT gives it directly); then
    o = h16 @ fp16(w2) + h16b @ fp16((w2 - fp16(w2)) * 2^10) accumulated in
    PSUM (scales cancel) corrects the w2 quantization: conv2 runs at fp16
    speed (~41us vs 82us fp32) adding only the h->fp16 rounding (~2 flips).
  - weights/x are cast to fp16 on the host with denormals flushed to zero so
    host and PE agree bit-exactly regardless of PE denormal handling.
  - ACT LUT churn: sigmoids for chunks 0,1 then their exps, then chunk 2's
    sig+exp pair; the e3-dependent DVE op is emitted last so the strict-FIFO
    DVE queue isn't blocked on the EXP table load in the kernel tail.
  - leaky_relu is ACT Prelu (alpha honored); Lrelu is a fixed-0.01 LUT.
  - score/label = max/argmax over sig(obj)*sig(cls) (the actual products, like
    the reference), via (s >= smax) * (1000 - idx) -> reduce_max -> 1000 - r,
    which matches jnp.argmax first-index tie behavior.
  - ~411us on quiet hardware = 3.42x over the fp32 baseline (1407us); PE is
    issue-saturated (conv1 ~347us at the 1 cyc/row fp16 roofline, conv2 ~42us,
    ~11us DMA-bootstrap head, ~10us tail/teardown). NOTE: sustained
    back-to-back benching drives the chip into P0 downclock (PE 2.4 -> 2.0
    GHz, everything x1.2); let it idle a minute before timing.
"""

import numpy as np

import concourse.bass as bass
import concourse.mybir as mybir
import concourse.tile as tile
from concourse import bacc
from concourse.bass_utils import run_bass_kernel_spmd

F32 = mybir.dt.float32
AF = mybir.ActivationFunctionType
ALU = mybir.AluOpType
AX = mybir.AxisListType

N_CORES = 8
B_PER = 8           # images per core
G = 19
HW = G * G          # 361
C_IN = 512
C_MID = 1024
NCI = 4             # ci chunks of 128
NCO = 8             # c_mid chunks of 128
NDET = 255
NANCH = 3
NCLS = 80
POS_CHUNKS = [(0, 128), (128, 128), (256, 105)]
OUT_FLOATS = HW * NANCH * 6  # 6498
BIG = 1000.0
# center tap first: it covers the full 19x19 output, so the PSUM accumulation
# group starts with a full overwrite; edge taps then accumulate partial
# regions. The dy-only taps (1, 7) go next: they read the unshifted x tile,
# deferring the taps that need the gpsimd-shifted copies by ~14us (image 0's
# copies land only after the x DMA + engine bootstrap)
TAP_ORDER = [4, 1, 7, 3, 5, 0, 2, 6, 8]


def build_nc():
    nc = bacc.Bacc()

    F16 = mybir.dt.float16
    xp = nc.dram_tensor("xp", [B_PER, NCI, 128, HW], F16, kind="ExternalInput")
    w1t = nc.dram_tensor("w1t", [36, 128, C_MID], F16, kind="ExternalInput")
    b1t = nc.dram_tensor("b1t", [128, NCO, 2], F32, kind="ExternalInput")
    w2t = nc.dram_tensor("w2t", [2, NCO, 128, NDET], F16, kind="ExternalInput")
    b2r = nc.dram_tensor("b2r", [NDET], F32, kind="ExternalInput")
    posc = nc.dram_tensor("posc", [128, 12], F32, kind="ExternalInput")
    iotw = nc.dram_tensor("iotw", [NCLS], F32, kind="ExternalInput")
    out = nc.dram_tensor("out", [B_PER, OUT_FLOATS], F32, kind="ExternalOutput")

    def bcast(ap_src, n):
        return bass.AP(tensor=ap_src.tensor, offset=ap_src.offset,
                       ap=[[0, n]] + [list(d) for d in ap_src.ap])

    with tile.TileContext(nc) as tc:
        with (
            tc.tile_pool(name="consts", bufs=1) as consts,
            tc.tile_pool(name="xpool", bufs=4) as xpool,
            tc.tile_pool(name="hpool", bufs=3) as hpool,
            tc.tile_pool(name="detpool", bufs=3) as detpool,
            tc.tile_pool(name="outpool", bufs=3) as outpool,
            tc.tile_pool(name="scratch", bufs=4) as scratch,
            tc.tile_pool(name="psum1", bufs=6, space="PSUM") as psum1,
            tc.tile_pool(name="psum2", bufs=2, space="PSUM") as psum2,
        ):
            # ---- image 0 input first (critical path), on the SWDGE queue so
            # it doesn't serialize behind the weight loads on sync's queue ----
            x0 = [xpool.tile([128, HW], F16, tag=f"x{c}", name=f"x0_{c}")
                  for c in range(NCI)]
            for c in range(NCI):
                nc.gpsimd.dma_start(out=x0[c], in_=xp[0, c])
            # small consts also on gpsimd (b1s is needed ~20us in)
            b1s = consts.tile([128, NCO, 2], F32, tag="b1s")
            nc.gpsimd.dma_start(out=b1s, in_=b1t[:, :, :])
            b2s = consts.tile([128, NDET], F32, tag="b2s")
            nc.gpsimd.dma_start(out=b2s, in_=bcast(b2r[:], 128))
            poss = consts.tile([128, 12], F32, tag="poss")
            nc.gpsimd.dma_start(out=poss, in_=posc[:, :])
            iots = consts.tile([128, NCLS], F32, tag="iots")
            nc.gpsimd.dma_start(out=iots, in_=bcast(iotw[:], 128))

            # ---- weights on sync, in consumption order; the very first
            # matmul's 64KB slice goes first so the PE can start ~2us sooner
            # than the full 512KB w1s[0] tile allows ----
            w10 = consts.tile([128, 128], F16, tag="w10")
            nc.sync.dma_start(out=w10, in_=w1t[0][:, 0:128])
            w1s = [consts.tile([128, C_MID], F16, tag=f"w1_{j}", name=f"w1_{j}")
                   for j in range(36)]
            for j in range(36):
                nc.sync.dma_start(out=w1s[j], in_=w1t[j])
            w2s = [[consts.tile([128, NDET], F16, tag=f"w2_{t}_{c}",
                                name=f"w2_{t}_{c}")
                    for c in range(NCO)] for t in range(2)]
            for t in range(2):
                for c in range(NCO):
                    nc.sync.dma_start(out=w2s[t][c], in_=w2t[t, c])

            # ---- HAM prewarm: dummy PE activity while the input DMAs spin up
            # (first data packets land ~9-11us in) releases the PE clock gate
            # (1.2 -> 2.4 GHz) before the first real matmul ----
            warm_src = scratch.tile([128, 256], mybir.dt.bfloat16, tag="warm")
            nc.vector.memset(warm_src, 0.0)
            wps = psum2.tile([128, 256], F32, tag="ps2", name="warmps")
            for _ in range(16):
                nc.tensor.matmul(wps, warm_src[:, :128], warm_src, start=True, stop=True)

            out_r = out.rearrange("b (p k) -> b p k", k=18)  # [B_PER, 361, 18]

            for b in range(B_PER):
                if b == 0:
                    xc = x0
                else:
                    xc = [xpool.tile([128, HW], F16, tag=f"x{c}", name=f"x{b}_{c}")
                          for c in range(NCI)]
                    for c in range(NCI):
                        nc.gpsimd.dma_start(out=xc[c], in_=xp[b, c])
                # column-shifted copies (zero edge col) so every conv tap reads
                # a fully contiguous window: strided moving APs cost ~6-17ns/mm
                # extra on the PE sequencer, 13us total across the kernel
                xsh = {}
                for c in range(NCI):
                    for dx in (-1, 1):
                        t = xpool.tile([128, HW], F16, tag=f"xs{c}_{dx}",
                                       name=f"xs{b}_{c}_{dx}")
                        tv = t.rearrange("p (h w) -> p h w", h=G)
                        xv = xc[c].rearrange("p (h w) -> p h w", h=G)
                        if dx == 1:  # t[y, x] = img[y, x+1], col 18 = 0
                            nc.gpsimd.memset(tv[:, :, G - 1], 0.0)
                            nc.gpsimd.tensor_copy(tv[:, :, 0:G - 1], xv[:, :, 1:G])
                        else:        # t[y, x] = img[y, x-1], col 0 = 0
                            nc.gpsimd.memset(tv[:, :, 0], 0.0)
                            nc.gpsimd.tensor_copy(tv[:, :, 1:G], xv[:, :, 0:G - 1])
                        xsh[(c, dx)] = t

                # ---- conv1: 3x3 valid-region accumulating matmuls ----
                # For image 0 the weights are still streaming in from HBM
                # (18.9MB at ~540GB/s vs oc-major consumption at ~870GB/s), so
                # run taps OUTER / oc INNER over 6 concurrent PSUM accumulators:
                # each arriving weight tile feeds 6x361 rows of PE work and the
                # PE never stalls on the weight stream. Later images use the
                # plain oc-major order (weights resident).
                h_t = [hpool.tile([128, NCO, HW], F16, tag=f"h{t}",
                                  name=f"h{b}_{t}")
                       for t in range(2)]
                jm_oc = 8 if b == 0 else 0
                if jm_oc:
                    # all 8 PSUM banks accumulate concurrently (conv2's 2 banks
                    # are idle during image 0): each arriving weight tile feeds
                    # 8 oc-chunks of rows, so the HBM weight stream stays ahead
                    pss = [psum1.tile([128, HW], F32, tag="ps1", name=f"ps1w{oc}")
                           for oc in range(6)]
                    pss += [psum2.tile([128, HW], F32, tag="ps2", name=f"ps2w{oc}")
                            for oc in range(6, jm_oc)]
                    psvs = [p.rearrange("p (h w) -> p h w", h=G) for p in pss]
                    for jj, tap in enumerate(TAP_ORDER):
                        ky, kx = divmod(tap, 3)
                        dy, dx = ky - 1, kx - 1
                        y0, ny = max(0, -dy), G - abs(dy)
                        for c in range(NCI):
                            xsrc = xc[c] if dx == 0 else xsh[(c, dx)]
                            xv = xsrc.rearrange("p (h w) -> p h w", h=G)
                            for oc in range(jm_oc):
                                lhsT = (w10 if (jj == 0 and c == 0 and oc == 0)
                                        else w1s[jj * NCI + c][:, oc * 128:(oc + 1) * 128])
                                nc.tensor.matmul(
                                    psvs[oc][:, y0:y0 + ny, :],
                                    lhsT,
                                    xv[:, y0 + dy:y0 + dy + ny, :],
                                    start=(jj == 0 and c == 0), stop=(jj == 8 and c == NCI - 1),
                                )
                    for oc in range(jm_oc):
                        for t in range(2):
                            nc.scalar.activation(
                                h_t[t][:, oc, :], pss[oc], AF.Prelu,
                                bias=b1s[:, oc, t:t + 1],
                                scale=(1.0 if t == 0 else 2.0 ** -10), alpha=0.1)
                for oc in range(jm_oc, NCO):
                    ps = psum1.tile([128, HW], F32, tag="ps1")
                    psv = ps.rearrange("p (h w) -> p h w", h=G)
                    k = 0
                    for jj, tap in enumerate(TAP_ORDER):
                        ky, kx = divmod(tap, 3)
                        dy, dx = ky - 1, kx - 1
                        y0, ny = max(0, -dy), G - abs(dy)
                        for c in range(NCI):
                            xsrc = xc[c] if dx == 0 else xsh[(c, dx)]
                            xv = xsrc.rearrange("p (h w) -> p h w", h=G)
                            nc.tensor.matmul(
                                psv[:, y0:y0 + ny, :],
                                w1s[jj * NCI + c][:, oc * 128:(oc + 1) * 128],
                                xv[:, y0 + dy:y0 + dy + ny, :],
                                start=(k == 0), stop=(k == 35),
                            )
                            k += 1
                    for t in range(2):
                        nc.scalar.activation(
                            h_t[t][:, oc, :], ps, AF.Prelu,
                            bias=b1s[:, oc, t:t + 1],
                            scale=(1.0 if t == 0 else 2.0 ** -10), alpha=0.1)

                # ---- conv2 (1x1, transposed out) + postprocess. ACT work is
                # batched per image (all sigmoids, then all exps) so the LUT
                # table is loaded twice per image instead of twice per chunk —
                # that churn otherwise sits on the critical-path tail ----
                chunks = []
                for pc, (p0, npos) in enumerate(POS_CHUNKS):
                    ps2 = psum2.tile([128, NDET], F32, tag="ps2")
                    for c in range(NCO):
                        for t in range(2):
                            nc.tensor.matmul(
                                ps2[:npos],
                                h_t[t][:, c, p0:p0 + npos],
                                w2s[t][c],
                                start=(c == 0 and t == 0),
                                stop=(c == NCO - 1 and t == 1),
                            )
                    det = detpool.tile([128, NDET], F32, tag="det")
                    nc.vector.tensor_tensor(det[:npos], ps2[:npos], b2s[:npos], op=ALU.add)

                    pstr = det.ap[0][0]
                    # [npos, 3, 5] view of the 5 box attrs per anchor
                    det5 = bass.AP(tensor=det.tensor, offset=det.offset,
                                   ap=[[pstr, npos], [85, NANCH], [1, 5]])
                    # [npos, 3, 80] view of the class logits per anchor
                    clsv = bass.AP(tensor=det.tensor, offset=det.offset + 5,
                                   ap=[[pstr, npos], [85, NANCH], [1, NCLS]])

                    sig5b = scratch.tile([128, NANCH, 5], F32, tag="sig5b")
                    sc3 = scratch.tile([128, NANCH, NCLS], F32, tag="sc3")
                    # scores = sig(obj) * sig(cls); score/label = max/argmax over
                    # the products, like the reference, so fp32 sigmoid
                    # saturation ties resolve identically
                    nc.scalar.activation(sig5b[:npos], det5, AF.Sigmoid)
                    nc.scalar.activation(sc3[:npos], clsv, AF.Sigmoid)
                    e3 = scratch.tile([128, NANCH, 2], F32, tag="e3")
                    chunks.append((pc, p0, npos, sig5b, sc3, e3))
                    # ACT LUT-churn control: EXP for chunks 0+1 runs right after
                    # their sigmoids (one table swap), the final chunk gets its
                    # own SIG->EXP pair so nothing queues behind the last conv2
                    if pc == 1:
                        for _, _, np_, s5, _, e in chunks:
                            nc.scalar.activation(e[:np_], s5[:np_, :, 3:5], AF.Exp)
                    elif pc == 2:
                        nc.scalar.activation(e3[:npos], sig5b[:npos, :, 3:5], AF.Exp)


                for pc, p0, npos, sig5b, sc3, e3 in chunks:
                    ot = outpool.tile([128, NANCH, 6], F32, tag="ot")
                    eq = scratch.tile([128, NANCH, NCLS], F32, tag="eq")
                    lm3 = scratch.tile([128, NANCH], F32, tag="lm3")
                    objb = bass.AP(tensor=sig5b.tensor, offset=sig5b.offset,
                                   ap=[[sig5b.ap[0][0], npos], [5, NANCH], [0, NCLS]])
                    nc.vector.tensor_tensor(sc3[:npos], sc3[:npos], objb, op=ALU.mult)
                    nc.vector.reduce_max(ot[:npos, :, 0], sc3[:npos], axis=AX.X)
                    # xc = sig(tx)/19 + gx/19 ; yc = sig(ty)/19 + gy/19 — on DVE
                    nc.vector.tensor_scalar(ot[:npos, :, 1], sig5b[:npos, :, 1],
                                            1.0 / G, poss[:npos, 2 * pc:2 * pc + 1],
                                            op0=ALU.mult, op1=ALU.add)
                    nc.vector.tensor_scalar(ot[:npos, :, 2], sig5b[:npos, :, 2],
                                            1.0 / G, poss[:npos, 2 * pc + 1:2 * pc + 2],
                                            op0=ALU.mult, op1=ALU.add)
                    # label = BIG - max((score >= max) * (BIG - idx)), first-index
                    # ties; emitted before the e3-dependent ops so the strict-FIFO
                    # DVE queue isn't blocked waiting for the EXP table load
                    smaxb = bass.AP(tensor=ot.tensor, offset=ot.offset,
                                    ap=[[ot.ap[0][0], npos], [6, NANCH], [0, NCLS]])
                    nc.vector.tensor_tensor(eq[:npos], sc3[:npos], smaxb, op=ALU.is_ge)
                    iotb = bass.AP(tensor=iots.tensor, offset=iots.offset,
                                   ap=[[iots.ap[0][0], npos], [0, NANCH], [1, NCLS]])
                    nc.vector.tensor_tensor(eq[:npos], eq[:npos], iotb, op=ALU.mult)
                    nc.vector.reduce_max(lm3[:npos], eq[:npos], axis=AX.X)
                    nc.vector.tensor_scalar(ot[:npos, :, 5], lm3[:npos], -1.0, BIG,
                                            op0=ALU.mult, op1=ALU.add)
                    # (w, h) = exp(sig(tw,th)) * anchors
                    anchv = bass.AP(tensor=poss.tensor, offset=poss.offset + 6,
                                    ap=[[poss.ap[0][0], npos], [2, NANCH], [1, 2]])
                    nc.vector.tensor_tensor(ot[:npos, :, 3:5], e3[:npos], anchv, op=ALU.mult)

                    nc.sync.dma_start(out=out_r[b, p0:p0 + npos, :], in_=ot[:npos])

    nc.finalize()
    return nc


_CACHE = {}


def _get_nc():
    if "nc" not in _CACHE:
        _CACHE["nc"] = build_nc()
    return _CACHE["nc"]


def _ftz16(a):
    """fp32 -> fp16 round-to-nearest with denormals flushed to zero (so host
    and PE agree bit-exactly regardless of the PE's denormal handling)."""
    h = a.astype(np.float16)
    h[np.abs(h) < 6.104e-05] = np.float16(0)  # < 2^-14 (fp16 min normal)
    return h


def _prep_inputs(x, conv_w, conv_b, detect_w, detect_b, anchors):
    # [core, b, ci_chunk, ci, 361] — pure reshape of the contiguous input
    xp = _ftz16(np.ascontiguousarray(x.reshape(N_CORES, B_PER, NCI, 128, HW)))
    # w1t[jj*4+c, ci, co] = conv_w[co, ci, ky, kx] with taps in TAP_ORDER
    w1t = _ftz16(np.ascontiguousarray(
        conv_w.transpose(2, 3, 1, 0).reshape(9, NCI, 128, C_MID)[TAP_ORDER]
        .reshape(36, 128, C_MID).astype(np.float32)))
    b1 = conv_b.reshape(NCO, 128).T.astype(np.float32)
    b1t = np.ascontiguousarray(
        np.stack([b1, b1 * np.float32(2.0 ** -10)], axis=-1))
    w2 = detect_w.reshape(NDET, C_MID).T.reshape(NCO, 128, NDET).astype(np.float32)
    w2hi = _ftz16(w2)
    w2lo = _ftz16((w2 - w2hi.astype(np.float32)) * np.float32(2.0 ** 10))
    w2t = np.ascontiguousarray(np.stack([w2hi, w2lo]))
    b2r = np.ascontiguousarray(detect_b.astype(np.float32))
    pos = np.arange(HW, dtype=np.float32)
    gx = (pos % G) / G
    gy = (pos // G).astype(np.float32) / G
    posc = np.zeros((128, 12), np.float32)
    for pc, (p0, npos) in enumerate(POS_CHUNKS):
        posc[:npos, 2 * pc] = gx[p0:p0 + npos]
        posc[:npos, 2 * pc + 1] = gy[p0:p0 + npos]
    posc[:, 6:12] = anchors.astype(np.float32).reshape(-1)[None, :]  # raw anchors
    iotw = (BIG - np.arange(NCLS, dtype=np.float32))
    return xp, w1t, b1t, w2t, b2r, posc, iotw


def kernel(x, conv_w, conv_b, detect_w, detect_b, anchors, _trace=False):
    x = np.asarray(x, np.float32)
    anchors = np.asarray(anchors, np.float32)
    nc = _get_nc()
    xp, w1t, b1t, w2t, b2r, posc, iotw = _prep_inputs(
        np.asarray(x, np.float32), np.asarray(conv_w, np.float32),
        np.asarray(conv_b, np.float32), np.asarray(detect_w, np.float32),
        np.asarray(detect_b, np.float32), anchors)
    shared = {"w1t": w1t, "b1t": b1t, "w2t": w2t, "b2r": b2r,
              "posc": posc, "iotw": iotw}
    in_maps = [{"xp": xp[c], **shared} for c in range(N_CORES)]
    res = run_bass_kernel_spmd(nc, in_maps, core_ids=list(range(N_CORES)),
                               trace=_trace)
    outs = np.stack([res.results[c]["out"] for c in range(N_CORES)])  # [8,8,6498]
    full = outs.reshape(64, HW * NANCH, 6)
    if _trace:
        return full, res
    return full

